# revision 17
# baseline (speedup 1.0000x reference)
"""Trainium2 Bass kernel for nn_NeuralNet_19250043421419.

Row-normalize x (mean/std over D=3072, ddof=1) then a 3-layer MLP
(3072->32->32->10) with LeakyReLU(0.01) after every layer.

Strategy: pure data parallel over 8 NeuronCores (batch 32768 -> 4096/core).
Rows are processed in blocks with graduated sizes (a 128-row opener,
512-row steady state, 256/128 closers) so the first block's compute
chain is exposed for only ~1 load unit and the last block's post-load
tail is short.

Load-path details (this kernel is HBM-bound; the x stream is the
wall-clock floor):
  - One SWDGE cast-DMA (fp32 -> fp16) per block.  Partition p receives
    ns consecutive DRAM rows (ns = nrows/128), so each descriptor
    covers ns*12KB of contiguous DRAM -- bigger descriptors, less
    per-descriptor overhead than the one-row-per-partition layout.
    Sub-tile s of the block is rows {p*ns + s}; every downstream stage
    uses the same permuted column order, and the output DMA's access
    pattern un-permutes (col q*128+p <-> row p*ns+q).
  - All constants ship as two packed blobs (f16 + f32) issued on the
    same SWDGE queue BEFORE the x loads: the queue drains FIFO so they
    are resident before the first x tile lands (HWDGE const loads
    starve ~20us behind the x stream's long packets).

Per block of N rows (ns = N/128 sub-tiles):
  - bn_stats/bn_aggr on DVE for per-row mean/var; one ACT rsqrt per
    block covers all sub-tiles; PE transposes mean/inv columns into
    [1, N] rows; fp16 row extraction on ACT.
  - PE transposes x into [d, i] tiles (fp16 matmuls vs identity), ACT
    copies PSUM->SBUF casting to fp16, and PE streams the transposed
    tiles against w1^T accumulating y0_raw in PSUM over 24 K-chunks
    (double-buffered in 1024-column PSUM groups).
  - Normalization is folded in afterwards: (x-m)/s @ w1^T =
    (y0_raw - m * rowsum(w1)) / s via a K=1 fp16 matmul into the same
    PSUM group and a DVE multiply against a partition-broadcast row.
  - Layers 2/3 are small matmuls in the transposed layout with
    per-partition ACT Lrelu bias APs.
  - The output is stored transposed ([10, B_CORE]) straight from the
    layer-3 activation tile and transposed on the host.
"""
import os
import sys

for _p in ("/opt/trn_rl_repo", "/root/.axon_site/_ro/trn_rl_repo"):
    if os.path.isdir(_p) and _p not in sys.path:
        sys.path.append(_p)

import numpy as np

import concourse.bass as bass
import concourse.bacc as bacc
import concourse.tile as tile
from concourse import mybir
from concourse.bass_utils import run_bass_kernel_spmd

F32 = mybir.dt.float32
F16 = mybir.dt.float16
AF = mybir.ActivationFunctionType

N_CORES = 8
B = 32768
D = 3072
H = 32
O = 10
B_CORE = B // N_CORES      # 4096
IBLK = 512                 # max rows per block
NSUB = IBLK // 128         # 4 sub-tiles of 128 rows max
NCHUNK = D // 128          # 24 contraction chunks
DDOF_SCALE = float(D) / float(D - 1)

# f16 blob column offsets
OFF_IDH = 0
OFF_W1 = 128               # NCHUNK chunks of [128, H]
OFF_W2 = OFF_W1 + NCHUNK * H    # [H, H] at partitions 0..31
OFF_W3 = OFF_W2 + H             # [H, O] at partitions 0..31
OFF_NEG = OFF_W3 + O            # [1, H]
CF16 = OFF_NEG + H
# f32 blob column offsets
OFF_IDF = 0
OFF_B1 = 128               # [H, 1]
OFF_B2 = 129               # [H, 1]
OFF_B3 = 130               # [O, 1]
CF32 = 131

# Graduated block sizes: short exposed chain at start and end, 512-row
# steady state in the middle.  Sums to B_CORE.
BLOCKS = [128] + [512] * 7 + [256, 128]
assert sum(BLOCKS) == B_CORE

LAST_EXEC_NS = None
_CACHE = {}


def _build():
    nc = bacc.Bacc("TRN2", target_bir_lowering=False, debug=False, num_devices=1)

    x_d = nc.dram_tensor("x", [B_CORE, D], F32, kind="ExternalInput").ap()
    cf16_d = nc.dram_tensor("cf16", [128, CF16], F16, kind="ExternalInput").ap()
    cf32_d = nc.dram_tensor("cf32", [128, CF32], F32, kind="ExternalInput").ap()
    y_d = nc.dram_tensor("y", [O, B_CORE], F32, kind="ExternalOutput").ap()

    with tile.TileContext(nc) as tc:
        with tc.tile_pool(name="consts", bufs=1) as consts, \
             tc.tile_pool(name="xpool", bufs=1) as xpool, \
             tc.tile_pool(name="xtpool", bufs=6) as xtpool, \
             tc.tile_pool(name="spool", bufs=4) as spool, \
             tc.tile_pool(name="pxt", bufs=2, space="PSUM") as pxt_pool, \
             tc.tile_pool(name="py0", bufs=2, space="PSUM") as py0_pool, \
             tc.tile_pool(name="pl", bufs=2, space="PSUM") as pl_pool:

            # ---- constants: two SWDGE DMAs on the x-load queue, issued
            # first so FIFO ordering lands them before any x tile ----
            cf16 = consts.tile([128, CF16], F16)
            nc.gpsimd.dma_start(out=cf16, in_=cf16_d)
            cf32 = consts.tile([128, CF32], F32)
            nc.gpsimd.dma_start(out=cf32, in_=cf32_d)

            idh_sb = cf16[:, OFF_IDH:OFF_IDH + 128]
            w2_sb = cf16[0:H, OFF_W2:OFF_W2 + H]
            w3_sb = cf16[0:H, OFF_W3:OFF_W3 + O]
            negs_sb = cf16[0:1, OFF_NEG:OFF_NEG + H]
            idf_sb = cf32[:, OFF_IDF:OFF_IDF + 128]
            b1_sb = cf32[0:H, OFF_B1:OFF_B1 + 1]
            b2_sb = cf32[0:H, OFF_B2:OFF_B2 + 1]
            b3_sb = cf32[0:O, OFF_B3:OFF_B3 + 1]

            def w1_ap(c):
                return cf16[:, OFF_W1 + c * H:OFF_W1 + (c + 1) * H]

            # ---- ACT table warm-up while the engines wait for x ----
            warm = spool.tile([H, 1], F32, tag="warm")
            nc.scalar.activation(warm, b2_sb, AF.Abs_reciprocal_sqrt, scale=1.0)
            nc.scalar.activation(warm, b2_sb, AF.Prelu, bias=b2_sb, scale=1.0,
                                 alpha=0.01)
            nc.scalar.copy(warm, b2_sb)

            def emit_tail(py0, inv_b, r0, nrows):
                # ---- normalize + layer 1 activation ----
                t1 = spool.tile([H, IBLK], F32, tag="t1")
                nc.vector.tensor_mul(t1[:, :nrows], py0[:, :nrows],
                                     inv_b[:, :nrows])
                h1 = spool.tile([H, IBLK], F16, tag="h1")
                nc.scalar.activation(h1[:, :nrows], t1[:, :nrows], AF.Prelu,
                                     bias=b1_sb, scale=1.0, alpha=0.01)
                # ---- layers 2 and 3 (small matmuls) ----
                p2 = pl_pool.tile([H, IBLK], F32, tag="pl")
                nc.tensor.matmul(p2[:, :nrows], w2_sb, h1[:, :nrows],
                                 start=True, stop=True)
                h2 = spool.tile([H, IBLK], F16, tag="h2")
                nc.scalar.activation(h2[:, :nrows], p2[:, :nrows], AF.Prelu,
                                     bias=b2_sb, scale=1.0, alpha=0.01)
                p3 = pl_pool.tile([O, IBLK], F32, tag="pl")
                nc.tensor.matmul(p3[:, :nrows], w3_sb, h2[:, :nrows],
                                 start=True, stop=True)
                y3 = spool.tile([O, IBLK], F32, tag="y3")
                nc.scalar.activation(y3[:, :nrows], p3[:, :nrows], AF.Prelu,
                                     bias=b3_sb, scale=1.0, alpha=0.01)
                # ---- store transposed (host un-permutes) ----
                nc.sync.dma_start(
                    out=y_d[:, r0:r0 + nrows], in_=y3[:, :nrows],
                )

            pending_tail = None
            r0 = 0
            for nrows in BLOCKS:
                ns = nrows // 128          # sub-tiles / rows per partition
                g = 1024 // nrows          # chunks per 1024-col PSUM group
                ngroups = NCHUNK // g

                # ---- load x block: cast-DMAs with ns consecutive DRAM
                # rows per partition (big contiguous descriptors), two
                # row-slots per DMA to keep completion granularity fine ----
                x_blk = x_d[r0:r0 + nrows, :].rearrange(
                    "(p q) d -> p q d", q=ns)
                xs = []
                for h in range(0, ns, 2):
                    w = min(2, ns - h)
                    xp = xpool.tile([128, 2, D], F16, tag="x2", bufs=7)
                    nc.gpsimd.dma_start(
                        out=xp[:, :w, :], in_=x_blk[:, h:h + w, :]
                    )
                    for j in range(w):
                        xs.append(xp[:, j, :])

                # ---- per-row stats on DVE; one rsqrt per block on ACT ----
                mvall = spool.tile([128, 2, NSUB], F32, tag="mv")
                for s in range(ns):
                    st6 = spool.tile([128, 6, 6], F32, tag="st6")
                    for k in range(6):
                        nc.vector.bn_stats(
                            out=st6[:, k, :], in_=xs[s][:, k * 512:(k + 1) * 512]
                        )
                    nc.vector.bn_aggr(out=mvall[:, :, s], in_=st6)
                invall = spool.tile([128, NSUB], F32, tag="invc")
                nc.scalar.activation(invall[:, :ns], mvall[:, 1, :ns],
                                     AF.Abs_reciprocal_sqrt, scale=DDOF_SCALE)

                # ---- stats to row layout: [128,1] cols -> [1, nrows] ----
                pmean = pl_pool.tile([1, IBLK], F32, tag="pl")
                pinv = pl_pool.tile([1, IBLK], F32, tag="pl")
                for s in range(ns):
                    nc.tensor.transpose(
                        pmean[:, s * 128:(s + 1) * 128], mvall[:, 0, s:s + 1],
                        idf_sb
                    )
                    nc.tensor.transpose(
                        pinv[:, s * 128:(s + 1) * 128],
                        invall[:, s:s + 1], idf_sb
                    )
                mean_row = spool.tile([1, IBLK], F16, tag="mrow")
                nc.scalar.copy(mean_row[:, :nrows], pmean[0:1, :nrows])
                inv_row = spool.tile([1, IBLK], F16, tag="irow")
                nc.scalar.copy(inv_row[:, :nrows], pinv[0:1, :nrows])
                inv_b = spool.tile([H, IBLK], F16, tag="invb")
                nc.gpsimd.partition_broadcast(inv_b[:, :nrows],
                                              inv_row[:, :nrows])

                # ---- transpose x (fp16 matmuls vs identity) + stream
                # against w1t ----
                py0 = py0_pool.tile([H, IBLK], F32)
                prev = None
                for G in range(ngroups):
                    pxt = pxt_pool.tile([128, 1024], F32)
                    for j in range(g):
                        c = G * g + j
                        for s in range(ns):
                            nc.tensor.matmul(
                                pxt[:, j * nrows + s * 128:
                                    j * nrows + (s + 1) * 128],
                                xs[s][:, c * 128:(c + 1) * 128],
                                idh_sb,
                                start=True, stop=True,
                            )
                    xts = xtpool.tile([128, 1024], F16, tag="xt")
                    nc.scalar.copy(xts, pxt)
                    if prev is not None:
                        pG, pxts = prev
                        for j in range(g):
                            c = pG * g + j
                            nc.tensor.matmul(
                                py0[:, :nrows], w1_ap(c),
                                pxts[:, j * nrows:(j + 1) * nrows],
                                start=(c == 0), stop=False,
                            )
                    prev = (G, xts)
                pG, pxts = prev
                for j in range(g):
                    c = pG * g + j
                    nc.tensor.matmul(
                        py0[:, :nrows], w1_ap(c),
                        pxts[:, j * nrows:(j + 1) * nrows],
                        start=False, stop=False,
                    )
                # mean correction: y0 -= rowsum(w1) (x) mean  (K=1 matmul)
                nc.tensor.matmul(py0[:, :nrows], negs_sb, mean_row[:, :nrows],
                                 start=False, stop=True)

                # ---- software-pipelined tail: emit the PREVIOUS block's
                # normalize/layers/store so the PE queue interleaves this
                # block's transpose groups ahead of the previous tail ----
                if pending_tail is not None:
                    emit_tail(*pending_tail)
                pending_tail = (py0, inv_b, r0, nrows)
                r0 += nrows

            emit_tail(*pending_tail)

    nc.compile()
    return nc


def _prep_inputs(x, w1, b1, w2, b2, w3, b3):
    x = np.ascontiguousarray(np.asarray(x, dtype=np.float32))
    w1 = np.asarray(w1, dtype=np.float32)
    w2 = np.asarray(w2, dtype=np.float32)
    w3 = np.asarray(w3, dtype=np.float32)
    b1 = np.asarray(b1, dtype=np.float32)
    b2 = np.asarray(b2, dtype=np.float32)
    b3 = np.asarray(b3, dtype=np.float32)

    cf16 = np.zeros((128, CF16), dtype=np.float16)
    cf16[:, OFF_IDH:OFF_IDH + 128] = np.eye(128, dtype=np.float16)
    w1t = w1.T.astype(np.float16)          # [D, H]
    for c in range(NCHUNK):
        cf16[:, OFF_W1 + c * H:OFF_W1 + (c + 1) * H] = \
            w1t[c * 128:(c + 1) * 128, :]
    cf16[0:H, OFF_W2:OFF_W2 + H] = w2.T.astype(np.float16)
    cf16[0:H, OFF_W3:OFF_W3 + O] = w3.T.astype(np.float16)
    cf16[0, OFF_NEG:OFF_NEG + H] = \
        (-w1.astype(np.float64).sum(axis=1)).astype(np.float16)

    cf32 = np.zeros((128, CF32), dtype=np.float32)
    cf32[:, OFF_IDF:OFF_IDF + 128] = np.eye(128, dtype=np.float32)
    cf32[0:H, OFF_B1] = b1
    cf32[0:H, OFF_B2] = b2
    cf32[0:O, OFF_B3] = b3

    common = {"cf16": cf16, "cf32": cf32}
    in_maps = []
    for c in range(N_CORES):
        m = dict(common)
        m["x"] = x[c * B_CORE:(c + 1) * B_CORE]
        in_maps.append(m)
    return in_maps


def kernel(x, w1, b1, w2, b2, w3, b3):
    global LAST_EXEC_NS
    if "nc" not in _CACHE:
        _CACHE["nc"] = _build()
    nc = _CACHE["nc"]
    in_maps = _prep_inputs(x, w1, b1, w2, b2, w3, b3)
    trace = bool(int(os.environ.get("KERNEL_PROFILE", "0")))
    res = run_bass_kernel_spmd(nc, in_maps, core_ids=list(range(N_CORES)),
                               trace=trace)
    LAST_EXEC_NS = res.exec_time_ns
    parts = []
    for r in res.results:
        yt = np.asarray(r["y"])          # [O, B_CORE], block-permuted cols
        yn = np.empty_like(yt)
        r0 = 0
        for nrows in BLOCKS:
            ns = nrows // 128
            seg = yt[:, r0:r0 + nrows].reshape(O, ns, 128)
            yn[:, r0:r0 + nrows] = seg.transpose(0, 2, 1).reshape(O, nrows)
            r0 += nrows
        parts.append(np.ascontiguousarray(yn.T))
    return np.concatenate(parts, axis=0).astype(np.float32)


# revision 19
# speedup vs baseline: 1.0079x; 1.0079x over previous
"""Trainium2 Bass kernel for nn_NeuralNet_19250043421419.

Row-normalize x (mean/std over D=3072, ddof=1) then a 3-layer MLP
(3072->32->32->10) with LeakyReLU(0.01) after every layer.

Strategy: pure data parallel over 8 NeuronCores (batch 32768 -> 4096/core).
Rows are processed in blocks with graduated sizes (a 128-row opener,
512-row steady state, 256/128 closers) so the first block's compute
chain is exposed for only ~1 load unit and the last block's post-load
tail is short.

Load-path details (this kernel is HBM-bound; the x stream is the
wall-clock floor):
  - One SWDGE cast-DMA (fp32 -> fp16) per block.  Partition p receives
    ns consecutive DRAM rows (ns = nrows/128), so each descriptor
    covers ns*12KB of contiguous DRAM -- bigger descriptors, less
    per-descriptor overhead than the one-row-per-partition layout.
    Sub-tile s of the block is rows {p*ns + s}; every downstream stage
    uses the same permuted column order, and the output DMA's access
    pattern un-permutes (col q*128+p <-> row p*ns+q).
  - All constants ship as two packed blobs (f16 + f32) issued on the
    same SWDGE queue BEFORE the x loads: the queue drains FIFO so they
    are resident before the first x tile lands (HWDGE const loads
    starve ~20us behind the x stream's long packets).

Per block of N rows (ns = N/128 sub-tiles):
  - bn_stats/bn_aggr on DVE for per-row mean/var; one ACT rsqrt per
    block covers all sub-tiles; PE transposes mean/inv columns into
    [1, N] rows; fp16 row extraction on ACT.
  - PE transposes x into [d, i] tiles (fp16 matmuls vs identity), ACT
    copies PSUM->SBUF casting to fp16, and PE streams the transposed
    tiles against w1^T accumulating y0_raw in PSUM over 24 K-chunks
    (double-buffered in 1024-column PSUM groups).
  - Normalization is folded in afterwards: (x-m)/s @ w1^T =
    (y0_raw - m * rowsum(w1)) / s via a K=1 fp16 matmul into the same
    PSUM group and a DVE multiply against a partition-broadcast row.
  - Layers 2/3 are small matmuls in the transposed layout with
    per-partition ACT Lrelu bias APs.
  - The output is stored transposed ([10, B_CORE]) straight from the
    layer-3 activation tile and transposed on the host.
"""
import os
import sys

for _p in ("/opt/trn_rl_repo", "/root/.axon_site/_ro/trn_rl_repo"):
    if os.path.isdir(_p) and _p not in sys.path:
        sys.path.append(_p)

import numpy as np

import concourse.bass as bass
import concourse.bacc as bacc
import concourse.tile as tile
from concourse import mybir
from concourse.bass_utils import run_bass_kernel_spmd

F32 = mybir.dt.float32
F16 = mybir.dt.float16
AF = mybir.ActivationFunctionType

N_CORES = 8
B = 32768
D = 3072
H = 32
O = 10
B_CORE = B // N_CORES      # 4096
IBLK = 512                 # max rows per block
NSUB = IBLK // 128         # 4 sub-tiles of 128 rows max
NCHUNK = D // 128          # 24 contraction chunks
DDOF_SCALE = float(D) / float(D - 1)

# f16 blob column offsets
OFF_IDH = 0
OFF_W1 = 128               # NCHUNK chunks of [128, H]
OFF_W2 = OFF_W1 + NCHUNK * H    # [H, H] at partitions 0..31
OFF_W3 = OFF_W2 + H             # [H, O] at partitions 0..31
OFF_NEG = OFF_W3 + O            # [1, H]
CF16 = OFF_NEG + H
# f32 blob column offsets
OFF_IDF = 0
OFF_B1 = 128               # [H, 1]
OFF_B2 = 129               # [H, 1]
OFF_B3 = 130               # [O, 1]
CF32 = 131

# Graduated block sizes: short exposed chain at start and end, 512-row
# steady state in the middle.  Sums to B_CORE.
BLOCKS = [128] + [512] * 7 + [256, 128]
assert sum(BLOCKS) == B_CORE

LAST_EXEC_NS = None
_CACHE = {}


def _build():
    nc = bacc.Bacc("TRN2", target_bir_lowering=False, debug=False, num_devices=1)

    x_d = nc.dram_tensor("x", [B_CORE, D], F32, kind="ExternalInput").ap()
    cf16_d = nc.dram_tensor("cf16", [128, CF16], F16, kind="ExternalInput").ap()
    cf32_d = nc.dram_tensor("cf32", [128, CF32], F32, kind="ExternalInput").ap()
    y_d = nc.dram_tensor("y", [O, B_CORE], F32, kind="ExternalOutput").ap()

    with tile.TileContext(nc) as tc:
        with tc.tile_pool(name="consts", bufs=1) as consts, \
             tc.tile_pool(name="xpool", bufs=1) as xpool, \
             tc.tile_pool(name="xtpool", bufs=6) as xtpool, \
             tc.tile_pool(name="spool", bufs=4) as spool, \
             tc.tile_pool(name="pxt", bufs=2, space="PSUM") as pxt_pool, \
             tc.tile_pool(name="py0", bufs=2, space="PSUM") as py0_pool, \
             tc.tile_pool(name="pl", bufs=2, space="PSUM") as pl_pool:

            # ---- constants: two SWDGE DMAs on the x-load queue, issued
            # first so FIFO ordering lands them before any x tile ----
            cf16 = consts.tile([128, CF16], F16)
            nc.gpsimd.dma_start(out=cf16, in_=cf16_d)
            cf32 = consts.tile([128, CF32], F32)
            nc.gpsimd.dma_start(out=cf32, in_=cf32_d)

            idh_sb = cf16[:, OFF_IDH:OFF_IDH + 128]
            w2_sb = cf16[0:H, OFF_W2:OFF_W2 + H]
            w3_sb = cf16[0:H, OFF_W3:OFF_W3 + O]
            negs_sb = cf16[0:1, OFF_NEG:OFF_NEG + H]
            idf_sb = cf32[:, OFF_IDF:OFF_IDF + 128]
            b1_sb = cf32[0:H, OFF_B1:OFF_B1 + 1]
            b2_sb = cf32[0:H, OFF_B2:OFF_B2 + 1]
            b3_sb = cf32[0:O, OFF_B3:OFF_B3 + 1]

            def w1_ap(c):
                return cf16[:, OFF_W1 + c * H:OFF_W1 + (c + 1) * H]

            # ---- ACT table warm-up while the engines wait for x ----
            warm = spool.tile([H, 1], F32, tag="warm")
            nc.scalar.activation(warm, b2_sb, AF.Abs_reciprocal_sqrt, scale=1.0)
            nc.scalar.activation(warm, b2_sb, AF.Prelu, bias=b2_sb, scale=1.0,
                                 alpha=0.01)
            nc.scalar.copy(warm, b2_sb)

            def emit_tail(h1, r0, nrows):
                # ---- layers 2 and 3 (small matmuls) ----
                p2 = pl_pool.tile([H, IBLK], F32, tag="pl")
                nc.tensor.matmul(p2[:, :nrows], w2_sb, h1[:, :nrows],
                                 start=True, stop=True)
                h2 = spool.tile([H, IBLK], F16, tag="h2")
                nc.scalar.activation(h2[:, :nrows], p2[:, :nrows], AF.Prelu,
                                     bias=b2_sb, scale=1.0, alpha=0.01)
                p3 = pl_pool.tile([O, IBLK], F32, tag="pl")
                nc.tensor.matmul(p3[:, :nrows], w3_sb, h2[:, :nrows],
                                 start=True, stop=True)
                y3 = spool.tile([O, IBLK], F32, tag="y3")
                nc.scalar.activation(y3[:, :nrows], p3[:, :nrows], AF.Prelu,
                                     bias=b3_sb, scale=1.0, alpha=0.01)
                # ---- store transposed (host un-permutes) ----
                nc.sync.dma_start(
                    out=y_d[:, r0:r0 + nrows], in_=y3[:, :nrows],
                )

            pending_tail = None
            r0 = 0
            for nrows in BLOCKS:
                ns = nrows // 128          # sub-tiles / rows per partition
                g = 1024 // nrows          # chunks per 1024-col PSUM group
                ngroups = NCHUNK // g

                # ---- load x block: cast-DMAs with ns consecutive DRAM
                # rows per partition (big contiguous descriptors), two
                # row-slots per DMA to keep completion granularity fine ----
                x_blk = x_d[r0:r0 + nrows, :].rearrange(
                    "(p q) d -> p q d", q=ns)
                xs = []
                for h in range(0, ns, 2):
                    w = min(2, ns - h)
                    xp = xpool.tile([128, 2, D], F16, tag="x2", bufs=7)
                    nc.gpsimd.dma_start(
                        out=xp[:, :w, :], in_=x_blk[:, h:h + w, :]
                    )
                    for j in range(w):
                        xs.append(xp[:, j, :])

                # ---- per-row stats on DVE; one rsqrt per block on ACT ----
                mvall = spool.tile([128, 2, NSUB], F32, tag="mv")
                for s in range(ns):
                    st6 = spool.tile([128, 6, 6], F32, tag="st6")
                    for k in range(6):
                        nc.vector.bn_stats(
                            out=st6[:, k, :], in_=xs[s][:, k * 512:(k + 1) * 512]
                        )
                    nc.vector.bn_aggr(out=mvall[:, :, s], in_=st6)
                invall = spool.tile([128, NSUB], F32, tag="invc")
                nc.scalar.activation(invall[:, :ns], mvall[:, 1, :ns],
                                     AF.Abs_reciprocal_sqrt, scale=DDOF_SCALE)

                # ---- stats to row layout: [128,1] cols -> [1, nrows] ----
                pmean = pl_pool.tile([1, IBLK], F32, tag="pl")
                pinv = pl_pool.tile([1, IBLK], F32, tag="pl")
                for s in range(ns):
                    nc.tensor.transpose(
                        pmean[:, s * 128:(s + 1) * 128], mvall[:, 0, s:s + 1],
                        idf_sb
                    )
                    nc.tensor.transpose(
                        pinv[:, s * 128:(s + 1) * 128],
                        invall[:, s:s + 1], idf_sb
                    )
                mean_row = spool.tile([1, IBLK], F16, tag="mrow")
                nc.scalar.copy(mean_row[:, :nrows], pmean[0:1, :nrows])
                inv_row = spool.tile([1, IBLK], F16, tag="irow")
                nc.scalar.copy(inv_row[:, :nrows], pinv[0:1, :nrows])
                inv_b = spool.tile([H, IBLK], F16, tag="invb")
                nc.gpsimd.partition_broadcast(inv_b[:, :nrows],
                                              inv_row[:, :nrows])

                # ---- transpose x (fp16 matmuls vs identity) + stream
                # against w1t ----
                py0 = py0_pool.tile([H, IBLK], F32)
                prev = None
                for G in range(ngroups):
                    pxt = pxt_pool.tile([128, 1024], F32)
                    for j in range(g):
                        c = G * g + j
                        for s in range(ns):
                            nc.tensor.matmul(
                                pxt[:, j * nrows + s * 128:
                                    j * nrows + (s + 1) * 128],
                                xs[s][:, c * 128:(c + 1) * 128],
                                idh_sb,
                                start=True, stop=True,
                            )
                    xts = xtpool.tile([128, 1024], F16, tag="xt")
                    nc.scalar.copy(xts, pxt)
                    if prev is not None:
                        pG, pxts = prev
                        for j in range(g):
                            c = pG * g + j
                            nc.tensor.matmul(
                                py0[:, :nrows], w1_ap(c),
                                pxts[:, j * nrows:(j + 1) * nrows],
                                start=(c == 0), stop=False,
                            )
                    prev = (G, xts)
                pG, pxts = prev
                for j in range(g):
                    c = pG * g + j
                    nc.tensor.matmul(
                        py0[:, :nrows], w1_ap(c),
                        pxts[:, j * nrows:(j + 1) * nrows],
                        start=False, stop=False,
                    )
                # mean correction: y0 -= rowsum(w1) (x) mean  (K=1 matmul)
                nc.tensor.matmul(py0[:, :nrows], negs_sb, mean_row[:, :nrows],
                                 start=False, stop=True)

                # ---- normalize + layer 1 activation (inline) ----
                t1 = spool.tile([H, IBLK], F32, tag="t1")
                nc.vector.tensor_mul(t1[:, :nrows], py0[:, :nrows],
                                     inv_b[:, :nrows])
                h1 = spool.tile([H, IBLK], F16, tag="h1")
                nc.scalar.activation(h1[:, :nrows], t1[:, :nrows], AF.Prelu,
                                     bias=b1_sb, scale=1.0, alpha=0.01)

                # ---- software-pipelined layers 2/3: emit the PREVIOUS
                # block's layers/store so the PE queue interleaves the next
                # block's transpose groups ahead of them ----
                if pending_tail is not None:
                    emit_tail(*pending_tail)
                pending_tail = (h1, r0, nrows)
                r0 += nrows

            emit_tail(*pending_tail)

    nc.compile()
    return nc


def _prep_inputs(x, w1, b1, w2, b2, w3, b3):
    x = np.ascontiguousarray(np.asarray(x, dtype=np.float32))
    w1 = np.asarray(w1, dtype=np.float32)
    w2 = np.asarray(w2, dtype=np.float32)
    w3 = np.asarray(w3, dtype=np.float32)
    b1 = np.asarray(b1, dtype=np.float32)
    b2 = np.asarray(b2, dtype=np.float32)
    b3 = np.asarray(b3, dtype=np.float32)

    cf16 = np.zeros((128, CF16), dtype=np.float16)
    cf16[:, OFF_IDH:OFF_IDH + 128] = np.eye(128, dtype=np.float16)
    w1t = w1.T.astype(np.float16)          # [D, H]
    for c in range(NCHUNK):
        cf16[:, OFF_W1 + c * H:OFF_W1 + (c + 1) * H] = \
            w1t[c * 128:(c + 1) * 128, :]
    cf16[0:H, OFF_W2:OFF_W2 + H] = w2.T.astype(np.float16)
    cf16[0:H, OFF_W3:OFF_W3 + O] = w3.T.astype(np.float16)
    cf16[0, OFF_NEG:OFF_NEG + H] = \
        (-w1.astype(np.float64).sum(axis=1)).astype(np.float16)

    cf32 = np.zeros((128, CF32), dtype=np.float32)
    cf32[:, OFF_IDF:OFF_IDF + 128] = np.eye(128, dtype=np.float32)
    cf32[0:H, OFF_B1] = b1
    cf32[0:H, OFF_B2] = b2
    cf32[0:O, OFF_B3] = b3

    common = {"cf16": cf16, "cf32": cf32}
    in_maps = []
    for c in range(N_CORES):
        m = dict(common)
        m["x"] = x[c * B_CORE:(c + 1) * B_CORE]
        in_maps.append(m)
    return in_maps


def kernel(x, w1, b1, w2, b2, w3, b3):
    global LAST_EXEC_NS
    if "nc" not in _CACHE:
        _CACHE["nc"] = _build()
    nc = _CACHE["nc"]
    in_maps = _prep_inputs(x, w1, b1, w2, b2, w3, b3)
    trace = bool(int(os.environ.get("KERNEL_PROFILE", "0")))
    res = run_bass_kernel_spmd(nc, in_maps, core_ids=list(range(N_CORES)),
                               trace=trace)
    LAST_EXEC_NS = res.exec_time_ns
    parts = []
    for r in res.results:
        yt = np.asarray(r["y"])          # [O, B_CORE], block-permuted cols
        yn = np.empty_like(yt)
        r0 = 0
        for nrows in BLOCKS:
            ns = nrows // 128
            seg = yt[:, r0:r0 + nrows].reshape(O, ns, 128)
            yn[:, r0:r0 + nrows] = seg.transpose(0, 2, 1).reshape(O, nrows)
            r0 += nrows
        parts.append(np.ascontiguousarray(yn.T))
    return np.concatenate(parts, axis=0).astype(np.float32)
